# revision 7
# baseline (speedup 1.0000x reference)
"""
nn_PerPointAttention — Trainium2 Bass kernel (8 NeuronCores, data-parallel).

Math (per batch b of 256):
    keys   = patches[b] @ Wk.T + patch_pos          [196, 128]
    vals   = patches[b] @ Wv.T                      [196, 128]
    logits = (queries @ keys.T)/sqrt(128)           [52, 196]
    logits = (logits + prior)/temp ; attn = softmax(logits, -1)
    out    = (attn @ vals) @ Wo.T + bo              [52, 1024]
Returns (out [256,52,1024] f32, attn [256,52,196] f32).

Strategy:
  - Pure data parallel: 32 batches per core, params replicated.
  - Host folds:  QkT = (queries@Wk).T/(scale*temp)  [1024,52]
                 Cfold = ((queries@pos.T)/scale + prior)/temp  [52,196]
    so on device:  logits = QkT.T @ patches.T + Cfold  (one E-contraction).
  - patches are cast to fp16 and DMA'd with the HW xbar transpose
    (2-byte dtype required) so patchesT [E,K] lands in SBUF with E on
    partitions — no on-chip transposes of the big tensor.  K padded
    196->208 (xbar needs src rows %16).
  - vals   = patchesT-as-lhsT @ WvT  (fp16, FWL weight loads)
  - attnT via two small PE transposes; attendedT = valsT@attnT;
    out = attendedT.T @ WoT (+bo) with batch-pairs packed into M.
"""

import math
from contextlib import ExitStack

import numpy as np

import concourse.bass as bass
import concourse.mybir as mybir
import concourse.tile as tile
from concourse import bacc
from concourse.masks import make_identity

F16 = mybir.dt.float16
F32 = mybir.dt.float32

B, K, E, A, P = 256, 196, 1024, 128, 52
NCORES = 8
NB = B // NCORES          # 32 batches per core
KP = 208                  # K padded to mult of 16 for xbar transpose
EC = E // 128             # 8 e-chunks
GROUP = 4                 # batches per transpose-DMA group (2 pairs)
SCALE = float(np.sqrt(128.0))

_cache = {}


def build_module(nb=NB, finalize=True):
    nc = bacc.Bacc("TRN2", target_bir_lowering=False, debug=False)

    patches_h = nc.dram_tensor("patches16", [nb, KP, E], F16, kind="ExternalInput")
    qkT_h = nc.dram_tensor("qkT", [128, EC, P], F16, kind="ExternalInput")
    wvT_h = nc.dram_tensor("wvT", [128, EC, A], F16, kind="ExternalInput")
    woT_h = nc.dram_tensor("woT", [A, E], F32, kind="ExternalInput")
    cfold_h = nc.dram_tensor("cfold", [P, K], F32, kind="ExternalInput")
    bo2_h = nc.dram_tensor("bo2", [2 * P, E], F32, kind="ExternalInput")

    out_h = nc.dram_tensor("out", [nb, P, E], F32, kind="ExternalOutput")
    attn_h = nc.dram_tensor("attn", [nb, P, K], F32, kind="ExternalOutput")

    pat2 = patches_h.rearrange("n k e -> (n k) e")     # [nb*KP, E]
    out2 = out_h.rearrange("n p e -> (n p) e")         # [nb*P, E]

    with tile.TileContext(nc) as tc, ExitStack() as ctx:
        singles = ctx.enter_context(tc.tile_pool(name="singles", bufs=1))
        qkT_sb = singles.tile([128, EC, P], F16)
        nc.gpsimd.dma_start(out=qkT_sb, in_=qkT_h[:])
        wvT_sb = singles.tile([128, EC, A], F16)
        nc.gpsimd.dma_start(out=wvT_sb, in_=wvT_h[:])
        woT_sb = singles.tile([A, E], F32)
        nc.gpsimd.dma_start(out=woT_sb, in_=woT_h[:])
        cfold_sb = singles.tile([P, K], F32)
        nc.gpsimd.dma_start(out=cfold_sb, in_=cfold_h[:])
        bo_sb = singles.tile([2 * P, E], F32)
        nc.gpsimd.dma_start(out=bo_sb, in_=bo2_h[:])
        ident = singles.tile([P, P], F32)
        make_identity(nc, ident)

        patT_pool = ctx.enter_context(tc.tile_pool(name="patT", bufs=2))
        lsb_pool = ctx.enter_context(tc.tile_pool(name="lsb", bufs=3))
        attn_pool = ctx.enter_context(tc.tile_pool(name="attnp", bufs=3))
        stat_pool = ctx.enter_context(tc.tile_pool(name="stats", bufs=4))
        attnT_pool = ctx.enter_context(tc.tile_pool(name="attnT", bufs=3))
        vals_pool = ctx.enter_context(tc.tile_pool(name="valsp", bufs=3))
        at_pool = ctx.enter_context(tc.tile_pool(name="atp", bufs=3))
        outp_pool = ctx.enter_context(tc.tile_pool(name="outp", bufs=2))

        psumL = ctx.enter_context(tc.tile_pool(name="psL", bufs=2, space="PSUM"))
        psumT = ctx.enter_context(tc.tile_pool(name="psT", bufs=1, space="PSUM"))
        psumV = ctx.enter_context(tc.tile_pool(name="psV", bufs=1, space="PSUM"))
        psumA = ctx.enter_context(tc.tile_pool(name="psA", bufs=1, space="PSUM"))
        psumO = ctx.enter_context(tc.tile_pool(name="psO", bufs=1, space="PSUM"))

        for g in range(nb // GROUP):
            patT = patT_pool.tile([128, EC, GROUP * KP], F16)
            for ec in range(EC):
                nc.sync.dma_start(
                    out=patT[:, ec, :],
                    in_=pat2[g * GROUP * KP:(g + 1) * GROUP * KP,
                             ec * 128:(ec + 1) * 128],
                    transpose=True,
                )
            for pr in range(GROUP // 2):
                b0 = g * GROUP + pr * 2          # global batch index (b0, b0+1)
                off = pr * 2 * KP                # col offset of the pair in patT

                # ---- logits for the pair: [52, 2*KP] ----
                Lp = psumL.tile([P, 2 * KP], F32)
                for ec in range(EC):
                    nc.tensor.matmul(
                        Lp, qkT_sb[:, ec, :], patT[:, ec, off:off + 2 * KP],
                        start=(ec == 0), stop=(ec == EC - 1),
                    )

                attn_sb = attn_pool.tile([P, 2, K], F32)
                Tp = psumT.tile([128, 2, 2, P], F32)          # [*, j, kc, P]
                attnT_sb = attnT_pool.tile([128, 2, 2, P], F32)
                vals_sb = vals_pool.tile([128, 2, 2, A], F32)  # [*, j, kc, a]
                Ap = psumA.tile([128, 2 * P], F32)

                for j in range(2):
                    # ---- softmax (no max-sub needed: logits <= ~0.2) ----
                    lsb = lsb_pool.tile([P, K], F32)
                    nc.vector.tensor_tensor(
                        out=lsb, in0=Lp[:, j * KP:j * KP + K], in1=cfold_sb,
                        op=mybir.AluOpType.add,
                    )
                    ssum = stat_pool.tile([P, 1], F32)
                    nc.scalar.activation(
                        attn_sb[:, j, :], lsb,
                        mybir.ActivationFunctionType.Exp,
                        accum_out=ssum,
                    )
                    rsum = stat_pool.tile([P, 1], F32)
                    nc.vector.reciprocal(rsum, ssum)
                    nc.vector.tensor_scalar_mul(attn_sb[:, j, :], attn_sb[:, j, :], rsum)
                    nc.gpsimd.dma_start(out=attn_h[b0 + j], in_=attn_sb[:, j, :])

                    # ---- attnT = attn.T  ([196,52] in two k-chunks) ----
                    nc.tensor.transpose(Tp[:, j, 0, :], attn_sb[:, j, 0:128], ident)
                    nc.tensor.transpose(Tp[0:68, j, 1, :], attn_sb[:, j, 128:196], ident)
                    nc.vector.tensor_copy(attnT_sb[:, j, 0, :], Tp[:, j, 0, :])
                    nc.scalar.copy(attnT_sb[0:68, j, 1, :], Tp[0:68, j, 1, :])

                for j in range(2):
                    boff = off + j * KP
                    # ---- vals = patchesT.T @ WvT  (k on partitions) ----
                    # two k-chunks accumulate concurrently -> separate banks
                    Vp0 = psumV.tile([128, A], F32)
                    Vp1 = psumV.tile([128, A], F32)
                    for ec in range(EC):
                        st, sp = (ec == 0), (ec == EC - 1)
                        nc.tensor.matmul(
                            Vp0, patT[:, ec, boff:boff + 128],
                            wvT_sb[:, ec, :], start=st, stop=sp,
                        )
                        nc.tensor.matmul(
                            Vp1[0:68, :], patT[:, ec, boff + 128:boff + K],
                            wvT_sb[:, ec, :], start=st, stop=sp,
                        )
                    nc.scalar.copy(vals_sb[:, j, 0, :], Vp0)
                    nc.scalar.copy(vals_sb[0:68, j, 1, :], Vp1[0:68, :])

                for j in range(2):
                    # ---- attendedT [128, 52] per batch, packed into Ap ----
                    nc.tensor.matmul(
                        Ap[:, j * P:(j + 1) * P], vals_sb[:, j, 0, :],
                        attnT_sb[:, j, 0, :], start=True, stop=False,
                    )
                    nc.tensor.matmul(
                        Ap[:, j * P:(j + 1) * P], vals_sb[0:68, j, 1, :],
                        attnT_sb[0:68, j, 1, :], start=False, stop=True,
                    )

                at_sb = at_pool.tile([128, 2 * P], F32)
                nc.vector.tensor_copy(at_sb, Ap)

                # ---- out pair [104, 1024] = at.T @ WoT + bo ----
                Op = psumO.tile([2 * P, E], F32)
                nc.tensor.matmul(Op[:, 0:512], at_sb, woT_sb[:, 0:512])
                nc.tensor.matmul(Op[:, 512:1024], at_sb, woT_sb[:, 512:1024])
                out_sb = outp_pool.tile([2 * P, E], F32)
                nc.vector.tensor_tensor(out=out_sb[:, 0:512], in0=Op[:, 0:512],
                                        in1=bo_sb[:, 0:512], op=mybir.AluOpType.add)
                nc.vector.tensor_tensor(out=out_sb[:, 512:1024], in0=Op[:, 512:1024],
                                        in1=bo_sb[:, 512:1024], op=mybir.AluOpType.add)
                nc.gpsimd.dma_start(out=out2[b0 * P:(b0 + 2) * P, :], in_=out_sb)

    if finalize:
        nc.finalize()
    return nc


def _prep_inputs(patches, queries, Wk, Wv, Wo, bo, patch_pos, temperature,
                 attn_prior, nb=NB, ncores=NCORES):
    patches = np.asarray(patches, np.float32)
    queries = np.asarray(queries, np.float32)
    Wk = np.asarray(Wk, np.float32)
    Wv = np.asarray(Wv, np.float32)
    Wo = np.asarray(Wo, np.float32)
    bo = np.asarray(bo, np.float32)
    patch_pos = np.asarray(patch_pos, np.float32)
    attn_prior = np.asarray(attn_prior, np.float32)
    t = float(np.asarray(temperature))
    temp = math.log1p(math.exp(t)) + 0.5 if t < 30 else t + 0.5

    QkT = ((queries @ Wk) / (SCALE * temp)).T            # [E, P]
    qkT_c = np.ascontiguousarray(
        QkT.reshape(EC, 128, P).transpose(1, 0, 2)).astype(np.float16)
    WvT = Wv.T                                            # [E, A]
    wvT_c = np.ascontiguousarray(
        WvT.reshape(EC, 128, A).transpose(1, 0, 2)).astype(np.float16)
    woT = np.ascontiguousarray(Wo.T)                      # [A, E]
    cfold = np.ascontiguousarray(
        ((queries @ patch_pos.T) / SCALE + attn_prior) / temp)  # [P, K]
    bo2 = np.ascontiguousarray(np.tile(bo[None, :], (2 * P, 1)))

    nuse = nb * ncores
    p16 = np.zeros((nuse, KP, E), np.float16)
    p16[:, :K, :] = patches[:nuse].astype(np.float16)
    shards = p16.reshape(ncores, nb, KP, E)

    in_maps = []
    for c in range(ncores):
        in_maps.append({
            "patches16": shards[c],
            "qkT": qkT_c, "wvT": wvT_c, "woT": woT,
            "cfold": cfold, "bo2": bo2,
        })
    return in_maps


def _get_module(nb=NB):
    if nb not in _cache:
        _cache[nb] = build_module(nb)
    return _cache[nb]


def _run(inputs, trace=False, nb=NB):
    from concourse.bass_utils import run_bass_kernel_spmd
    nc = _get_module(nb)
    in_maps = _prep_inputs(**inputs, nb=nb)
    res = run_bass_kernel_spmd(nc, in_maps, core_ids=list(range(NCORES)),
                               trace=trace)
    outs = np.concatenate([r["out"] for r in res.results], axis=0)
    attns = np.concatenate([r["attn"] for r in res.results], axis=0)
    return (outs, attns), res


def kernel(**inputs):
    (outs, attns), _ = _run(inputs, trace=False)
    return outs, attns


# revision 50
# speedup vs baseline: 480.4667x; 480.4667x over previous
"""
nn_PerPointAttention — Trainium2 Bass kernel (8 NeuronCores, data-parallel).

Math (per batch b of 256):
    keys   = patches[b] @ Wk.T + patch_pos          [196, 128]
    vals   = patches[b] @ Wv.T                      [196, 128]
    logits = (queries @ keys.T)/sqrt(128)           [52, 196]
    logits = (logits + prior)/temp ; attn = softmax(logits, -1)
    out    = (attn @ vals) @ Wo.T + bo              [52, 1024]
Returns (out [256,52,1024] f32, attn [256,52,196] f32).

Strategy:
  - Pure data parallel: 32 batches per core, params replicated.
  - Host folds:  QkT = (queries@Wk).T/(scale*temp)  [1024,52]
                 Cfold = ((queries@pos.T)/scale + prior)/temp  [52,196]
    so on device:  logits = QkT.T @ patches.T + Cfold  (one E-contraction).
  - patches are cast to fp16 and DMA'd with the HW xbar transpose
    (2-byte dtype required) so patchesT [E,K] lands in SBUF with E on
    partitions — no on-chip transposes of the big tensor.  K padded
    196->208 (xbar needs src rows %16).
  - vals   = patchesT-as-lhsT @ WvT  (fp16, FWL weight loads)
  - attnT via two small PE transposes; attendedT = valsT@attnT;
    out = attendedT.T @ WoT (+bo) with batch-pairs packed into M.
"""

import math
from contextlib import ExitStack

import numpy as np

import concourse.bass as bass
import concourse.mybir as mybir
import concourse.tile as tile
from concourse import bacc
from concourse.masks import make_identity

F16 = mybir.dt.float16
F32 = mybir.dt.float32

B, K, E, A, P = 256, 196, 1024, 128, 52
NCORES = 8
NB = B // NCORES          # 32 batches per core
KP = 196                  # no per-batch pad: group row counts are %16
EC = E // 128             # 8 e-chunks
B1 = 64                   # partition base of pair-batch1 (lane-split layout)
GROUP = 4                 # batches per transpose-DMA group (2 pairs)
SCALE = float(np.sqrt(128.0))

# pool depth knobs (overridable for experiments)
CFG = dict(patT_bufs=2, lsb_bufs=4, attn_bufs=4, stat_bufs=6, attnT_bufs=3,
           vals_bufs=3, at_bufs=3, outp_bufs=2,
           psL=2, psT=1, psV=2, psA=1, psO=1,
           group=16, attn_t_layout=True, tp_split=0, vals_copy_first=False,
           sched_head=())

_cache = {}


def build_module(nb=NB, finalize=True):
    nc = bacc.Bacc("TRN2", target_bir_lowering=False, debug=False)

    patches_h = nc.dram_tensor("patches16", [nb, KP, E], F16, kind="ExternalInput")
    qkT_h = nc.dram_tensor("qkT", [128, EC, B1], F16, kind="ExternalInput")
    wvT_h = nc.dram_tensor("wvT", [128, EC, A], F16, kind="ExternalInput")
    woT_h = nc.dram_tensor("woT", [A, E], F16, kind="ExternalInput")
    cfold_h = nc.dram_tensor("cfold", [2 * B1, K], F32, kind="ExternalInput")
    bo2_h = nc.dram_tensor("bo2", [2 * P, E], F32, kind="ExternalInput")

    out_h = nc.dram_tensor("out", [nb, P, E], F32, kind="ExternalOutput")
    # raw lane-split pair layout; host extracts rows 0:52 / 64:116 per pair
    attn_h = nc.dram_tensor("attn", [2 * B1, nb // 2, K], F32,
                            kind="ExternalOutput")

    pat2 = patches_h.rearrange("n k e -> (n k) e")     # [nb*KP, E]
    out2 = out_h.rearrange("n p e -> (n p) e")         # [nb*P, E]

    with tile.TileContext(nc) as tc, ExitStack() as ctx:
        singles = ctx.enter_context(tc.tile_pool(name="singles", bufs=1))
        qkT_sb = singles.tile([128, EC, B1], F16)
        nc.gpsimd.dma_start(out=qkT_sb, in_=qkT_h[:])
        wvT_sb = singles.tile([128, EC, A], F16)
        nc.gpsimd.dma_start(out=wvT_sb, in_=wvT_h[:])
        woT_sb = singles.tile([A, E], F16)
        nc.gpsimd.dma_start(out=woT_sb, in_=woT_h[:])
        cfold_sb = singles.tile([2 * B1, K], F32)
        nc.gpsimd.dma_start(out=cfold_sb, in_=cfold_h[:])
        bo_sb = singles.tile([2 * P, E], F32)
        nc.gpsimd.dma_start(out=bo_sb, in_=bo2_h[:])
        ident = singles.tile([B1 + P, P], F16)
        make_identity(nc, ident[0:P, :])
        make_identity(nc, ident[B1:B1 + P, :])

        # variable group schedule: small first groups to shorten the ramp
        rem, sched = nb, []
        for want in CFG["sched_head"]:
            if rem >= want and want <= CFG["group"]:
                sched.append(want)
                rem -= want
        while rem:
            take = min(CFG["group"], rem)
            sched.append(take)
            rem -= take
        gstart = [0]
        for sz in sched[:-1]:
            gstart.append(gstart[-1] + sz)
        pair2g = []
        for gi, sz in enumerate(sched):
            pair2g += [(gi, pr) for pr in range(sz // 2)]
        gp = max(sched)
        patT_pool = ctx.enter_context(tc.tile_pool(name="patT", bufs=CFG["patT_bufs"]))
        lsb_pool = ctx.enter_context(tc.tile_pool(name="lsb", bufs=CFG["lsb_bufs"]))
        attn_pool = ctx.enter_context(tc.tile_pool(name="attnp", bufs=CFG["attn_bufs"]))
        stat_pool = ctx.enter_context(tc.tile_pool(name="stats", bufs=CFG["stat_bufs"]))
        attnT_pool = ctx.enter_context(tc.tile_pool(name="attnT", bufs=CFG["attnT_bufs"]))
        vals_pool = ctx.enter_context(tc.tile_pool(name="valsp", bufs=CFG["vals_bufs"]))
        at_pool = ctx.enter_context(tc.tile_pool(name="atp", bufs=CFG["at_bufs"]))
        outp_pool = ctx.enter_context(tc.tile_pool(name="outp", bufs=CFG["outp_bufs"]))

        psumL = ctx.enter_context(tc.tile_pool(name="psL", bufs=CFG["psL"], space="PSUM"))
        psumT = ctx.enter_context(tc.tile_pool(name="psT", bufs=CFG["psT"], space="PSUM"))
        psumV = ctx.enter_context(tc.tile_pool(name="psV", bufs=CFG["psV"], space="PSUM"))
        psumA = ctx.enter_context(tc.tile_pool(name="psA", bufs=CFG["psA"], space="PSUM"))
        psumO = ctx.enter_context(tc.tile_pool(name="psO", bufs=CFG["psO"], space="PSUM"))

        # --- software-pipelined emission: per pair p, emit the PE "front"
        # (logits+vals, independent of softmax) and the softmax chain, but
        # delay the softmax-DEPENDENT PE tail (transposes/attended/out) of
        # pair p until after pair p+1's front is in the PE queue.  Engines
        # are strict FIFO, so this prevents head-of-line blocking.
        npairs = nb // 2

        def emit_loads(g):
            patTs = []
            rows = sched[g] * KP
            r0 = gstart[g] * KP
            for ec in range(EC):
                pt = patT_pool.tile([128, rows], F16, name=f"patT{ec}",
                                    tag=f"patT{ec}")
                eng = nc.scalar if ec < CFG["tp_split"] else nc.sync
                eng.dma_start(
                    out=pt,
                    in_=pat2[r0:r0 + rows, ec * 128:(ec + 1) * 128],
                    transpose=True,
                )
                patTs.append(pt)
            return patTs

        group_tiles = {}

        # Lane-split pair layout: batch0 of a pair lives on partitions 0:52,
        # batch1 on partitions 64:116 (32-aligned base for matmul outputs).
        # The whole softmax chain then runs ONCE per pair on [116, K].

        def emit_front(p):
            g, pr = pair2g[p]
            if g == 0 and pr == 0:
                group_tiles[0] = emit_loads(0)
            if pr == 0 and g + 1 < len(sched):
                group_tiles[g + 1] = emit_loads(g + 1)
            patTs = group_tiles[g]
            off = pr * 2 * KP
            st = {"patTs": patTs, "off": off, "b0": p * 2}
            Lp = psumL.tile([2 * B1, K], F32)
            for ec in range(EC):
                nc.tensor.matmul(
                    Lp[0:B1, :], qkT_sb[:, ec, :], patTs[ec][:, off:off + K],
                    start=(ec == 0), stop=(ec == EC - 1),
                )
            for ec in range(EC):
                nc.tensor.matmul(
                    Lp[B1:2 * B1, :], qkT_sb[:, ec, :],
                    patTs[ec][:, off + KP:off + KP + K],
                    start=(ec == 0), stop=(ec == EC - 1),
                    skip_group_check=True,
                )
            st["Lp"] = Lp
            st["vals_sb"] = vals_pool.tile([128, 2, 2, A], F16, name="vals_sb",
                                           tag="vals_sb")
            st["Vp"] = []
            for j in range(2):
                boff = off + j * KP
                # one bank; the two k-chunks run as SEQUENTIAL psum groups
                Vp = psumV.tile([128, 2, A], F32)
                for ec in range(EC):
                    nc.tensor.matmul(Vp[:, 0, :], patTs[ec][:, boff:boff + 128],
                                     wvT_sb[:, ec, :], start=(ec == 0),
                                     stop=(ec == EC - 1))
                for ec in range(EC):
                    nc.tensor.matmul(Vp[0:68, 1, :],
                                     patTs[ec][:, boff + 128:boff + K],
                                     wvT_sb[:, ec, :], start=(ec == 0),
                                     stop=(ec == EC - 1))
                st["Vp"].append(Vp)
            return st

        def emit_chain(st):
            Lp = st["Lp"]
            if CFG["vals_copy_first"]:
                vals_sb = st["vals_sb"]
                for j in range(2):
                    Vp = st["Vp"][j]
                    nc.scalar.copy(vals_sb[:, j, 0, :], Vp[:, 0, :])
                    nc.scalar.copy(vals_sb[0:68, j, 1, :], Vp[0:68, 1, :])
            lsb = lsb_pool.tile([2 * B1, K], F32)
            nc.vector.tensor_tensor(out=lsb, in0=Lp[:, 0:K], in1=cfold_sb,
                                    op=mybir.AluOpType.add)
            ssum = stat_pool.tile([2 * B1, 1], F32)
            attn_un = attn_pool.tile([2 * B1, K], F32, name="attn_un",
                                     tag="attn_un")
            nc.scalar.activation(attn_un, lsb,
                                 mybir.ActivationFunctionType.Exp,
                                 accum_out=ssum)
            rsum = stat_pool.tile([2 * B1, 1], F32)
            nc.vector.reciprocal(rsum, ssum)
            attn16 = attn_pool.tile([2 * B1, K], F16, name="attn16", tag="attn16")
            nc.vector.tensor_scalar_mul(attn16, attn_un, rsum)
            st["attn16"], st["rsum"] = attn16, rsum
            st["attn_un"] = attn_un
            attn_f32 = attn_pool.tile([2 * B1, K], F32, name="attn_f32",
                                      tag="attn_f32")
            nc.vector.tensor_scalar_mul(attn_f32, attn_un, rsum)
            nc.gpsimd.dma_start(out=attn_h[:, st["b0"] // 2, :], in_=attn_f32)
            if not CFG["vals_copy_first"]:
                vals_sb = st["vals_sb"]
                for j in range(2):
                    Vp = st["Vp"][j]
                    nc.scalar.copy(vals_sb[:, j, 0, :], Vp[:, 0, :])
                    nc.scalar.copy(vals_sb[0:68, j, 1, :], Vp[0:68, 1, :])

        def emit_tail(st):
            b0, attn16 = st["b0"], st["attn16"]
            vals_sb = st["vals_sb"]
            Tp = psumT.tile([128, 2, 2, P], F16)          # [*, j, kc, P]
            attnT_sb = attnT_pool.tile([128, 2, 2, P], F16)
            for j in range(2):
                jb = j * B1
                idn = ident[jb:jb + P, :]
                nc.tensor.transpose(Tp[:, j, 0, :],
                                    attn16[jb:jb + P, 0:128], idn)
                nc.tensor.transpose(Tp[0:68, j, 1, :],
                                    attn16[jb:jb + P, 128:196], idn)
                nc.vector.tensor_copy(attnT_sb[:, j, 0, :], Tp[:, j, 0, :])
                nc.scalar.copy(attnT_sb[0:68, j, 1, :], Tp[0:68, j, 1, :])
            Ap = psumA.tile([128, 2 * P], F32)
            for j in range(2):
                nc.tensor.matmul(Ap[:, j * P:(j + 1) * P], vals_sb[:, j, 0, :],
                                 attnT_sb[:, j, 0, :], start=True, stop=False)
                nc.tensor.matmul(Ap[:, j * P:(j + 1) * P], vals_sb[0:68, j, 1, :],
                                 attnT_sb[0:68, j, 1, :], start=False, stop=True)
            at_sb = at_pool.tile([128, 2 * P], F16)
            nc.vector.tensor_copy(at_sb, Ap)
            Op = psumO.tile([2 * P, E], F32)
            nc.tensor.matmul(Op[:, 0:512], at_sb, woT_sb[:, 0:512])
            nc.tensor.matmul(Op[:, 512:1024], at_sb, woT_sb[:, 512:1024])
            out_sb = outp_pool.tile([2 * P, E], F32)
            nc.vector.tensor_tensor(out=out_sb[:, 0:512], in0=Op[:, 0:512],
                                    in1=bo_sb[:, 0:512], op=mybir.AluOpType.add)
            nc.vector.tensor_tensor(out=out_sb[:, 512:1024], in0=Op[:, 512:1024],
                                    in1=bo_sb[:, 512:1024], op=mybir.AluOpType.add)
            nc.gpsimd.dma_start(out=out2[b0 * P:(b0 + 2) * P, :], in_=out_sb)

        pending = None
        for p in range(npairs):
            st = emit_front(p)
            emit_chain(st)
            if pending is not None:
                emit_tail(pending)
            pending = st
        emit_tail(pending)

    if finalize:
        nc.finalize()
    return nc


def _prep_inputs(patches, queries, Wk, Wv, Wo, bo, patch_pos, temperature,
                 attn_prior, nb=NB, ncores=NCORES):
    patches = np.asarray(patches, np.float32)
    queries = np.asarray(queries, np.float32)
    Wk = np.asarray(Wk, np.float32)
    Wv = np.asarray(Wv, np.float32)
    Wo = np.asarray(Wo, np.float32)
    bo = np.asarray(bo, np.float32)
    patch_pos = np.asarray(patch_pos, np.float32)
    attn_prior = np.asarray(attn_prior, np.float32)
    t = float(np.asarray(temperature))
    temp = math.log1p(math.exp(t)) + 0.5 if t < 30 else t + 0.5

    QkT = ((queries @ Wk) / (SCALE * temp)).T            # [E, P]
    QkT64 = np.zeros((E, B1), np.float32)                # pad P=52 -> 64 cols
    QkT64[:, :P] = QkT
    qkT_c = np.ascontiguousarray(
        QkT64.reshape(EC, 128, B1).transpose(1, 0, 2)).astype(np.float16)
    WvT = Wv.T                                            # [E, A]
    wvT_c = np.ascontiguousarray(
        WvT.reshape(EC, 128, A).transpose(1, 0, 2)).astype(np.float16)
    woT = np.ascontiguousarray(Wo.T).astype(np.float16)   # [A, E]
    cf = ((queries @ patch_pos.T) / SCALE + attn_prior) / temp  # [P, K]
    cfold = np.zeros((2 * B1, K), np.float32)                   # lane-split dup
    cfold[:P] = cf
    cfold[B1:B1 + P] = cf
    bo2 = np.ascontiguousarray(np.tile(bo[None, :], (2 * P, 1)))

    nuse = nb * ncores
    p16 = patches[:nuse].astype(np.float16)
    shards = p16.reshape(ncores, nb, KP, E)

    in_maps = []
    for c in range(ncores):
        in_maps.append({
            "patches16": shards[c],
            "qkT": qkT_c, "wvT": wvT_c, "woT": woT,
            "cfold": cfold, "bo2": bo2,
        })
    return in_maps


def _get_module(nb=NB):
    key = (nb, tuple(sorted(CFG.items())))
    if key not in _cache:
        _cache[key] = build_module(nb)
    return _cache[key]


def _run(inputs, trace=False, nb=NB):
    from concourse.bass_utils import run_bass_kernel_spmd
    nc = _get_module(nb)
    in_maps = _prep_inputs(**inputs, nb=nb)
    res = run_bass_kernel_spmd(nc, in_maps, core_ids=list(range(NCORES)),
                               trace=trace)
    outs = np.concatenate([r["out"] for r in res.results], axis=0)
    attns = np.concatenate([_attn_fix(r["attn"]) for r in res.results], axis=0)
    return (outs, attns), res


def _attn_fix(a):
    # a: [128, nb//2, K] lane-split pairs -> [nb, P, K]
    npr = a.shape[1]
    out = np.empty((npr * 2, P, K), a.dtype)
    out[0::2] = a[0:P].transpose(1, 0, 2)
    out[1::2] = a[B1:B1 + P].transpose(1, 0, 2)
    return out


def kernel(**inputs):
    (outs, attns), _ = _run(inputs, trace=False)
    return outs, attns


# revision 54
# speedup vs baseline: 868.1005x; 1.8068x over previous
"""
nn_PerPointAttention — Trainium2 Bass kernel (8 NeuronCores, data-parallel).

Math (per batch b of 256):
    keys   = patches[b] @ Wk.T + patch_pos          [196, 128]
    vals   = patches[b] @ Wv.T                      [196, 128]
    logits = (queries @ keys.T)/sqrt(128)           [52, 196]
    logits = (logits + prior)/temp ; attn = softmax(logits, -1)
    out    = (attn @ vals) @ Wo.T + bo              [52, 1024]
Returns (out [256,52,1024] f32, attn [256,52,196] f32).

Strategy:
  - Pure data parallel: 32 batches per core, params replicated.
  - Host folds:  QkT = (queries@Wk).T/(scale*temp)  [1024,52]
                 Cfold = ((queries@pos.T)/scale + prior)/temp  [52,196]
    so on device:  logits = QkT.T @ patches.T + Cfold  (one E-contraction).
  - patches are cast to fp16 and DMA'd with the HW xbar transpose
    (2-byte dtype required) so patchesT [E,K] lands in SBUF with E on
    partitions — no on-chip transposes of the big tensor.  K padded
    196->208 (xbar needs src rows %16).
  - vals   = patchesT-as-lhsT @ WvT  (fp16, FWL weight loads)
  - attnT via two small PE transposes; attendedT = valsT@attnT;
    out = attendedT.T @ WoT (+bo) with batch-pairs packed into M.
"""

import math
from contextlib import ExitStack

import numpy as np

import concourse.bass as bass
import concourse.mybir as mybir
import concourse.tile as tile
from concourse import bacc
from concourse.masks import make_identity

F16 = mybir.dt.float16
F32 = mybir.dt.float32

B, K, E, A, P = 256, 196, 1024, 128, 52
NCORES = 8
NB = B // NCORES          # 32 batches per core
KP = 196                  # no per-batch pad: group row counts are %16
EC = E // 128             # 8 e-chunks
B1 = 64                   # partition base of pair-batch1 (lane-split layout)
GROUP = 4                 # batches per transpose-DMA group (2 pairs)
SCALE = float(np.sqrt(128.0))

# pool depth knobs (overridable for experiments)
CFG = dict(patT_bufs=4, lsb_bufs=4, attn_bufs=4, stat_bufs=6, attnT_bufs=3,
           vals_bufs=3, at_bufs=3, outp_bufs=2,
           psL=2, psT=1, psV=1, psVt=1, psA=1, psO=1,
           group=4, attn_t_layout=True, tp_split=3, vals_copy_first=False,
           sched_head=(), patT_bufs_unused=0)

_cache = {}


def build_module(nb=NB, finalize=True):
    nc = bacc.Bacc("TRN2", target_bir_lowering=False, debug=False)

    patches_h = nc.dram_tensor("patches16", [EC, 128, nb * KP], F16,
                               kind="ExternalInput")
    qkT_h = nc.dram_tensor("qkT", [128, EC, B1], F16, kind="ExternalInput")
    wvT_h = nc.dram_tensor("wvT", [128, EC, A], F16, kind="ExternalInput")
    woT_h = nc.dram_tensor("woT", [A, E], F16, kind="ExternalInput")
    cfold_h = nc.dram_tensor("cfold", [2 * B1, K], F32, kind="ExternalInput")
    bo2_h = nc.dram_tensor("bo2", [2 * P, E], F32, kind="ExternalInput")

    out_h = nc.dram_tensor("out", [nb, P, E], F32, kind="ExternalOutput")
    # raw lane-split pair layout; host extracts rows 0:52 / 64:116 per pair
    attn_h = nc.dram_tensor("attn", [2 * B1, nb // 2, K], F32,
                            kind="ExternalOutput")


    out2 = out_h.rearrange("n p e -> (n p) e")         # [nb*P, E]

    with tile.TileContext(nc) as tc, ExitStack() as ctx:
        singles = ctx.enter_context(tc.tile_pool(name="singles", bufs=1))
        qkT_sb = singles.tile([128, EC, B1], F16)
        nc.gpsimd.dma_start(out=qkT_sb, in_=qkT_h[:])
        wvT_sb = singles.tile([128, EC, A], F16)
        nc.gpsimd.dma_start(out=wvT_sb, in_=wvT_h[:])
        woT_sb = singles.tile([A, E], F16)
        nc.gpsimd.dma_start(out=woT_sb, in_=woT_h[:])
        cfold_sb = singles.tile([2 * B1, K], F32)
        nc.gpsimd.dma_start(out=cfold_sb, in_=cfold_h[:])
        bo_sb = singles.tile([2 * P, E], F32)
        nc.gpsimd.dma_start(out=bo_sb, in_=bo2_h[:])
        ident = singles.tile([B1 + P, P], F16)
        make_identity(nc, ident[0:P, :])
        make_identity(nc, ident[B1:B1 + P, :])
        ident128 = singles.tile([128, 128], F16)
        make_identity(nc, ident128)

        # variable group schedule: small first groups to shorten the ramp
        rem, sched = nb, []
        for want in CFG["sched_head"]:
            if rem >= want and want <= CFG["group"]:
                sched.append(want)
                rem -= want
        while rem:
            take = min(CFG["group"], rem)
            sched.append(take)
            rem -= take
        gstart = [0]
        for sz in sched[:-1]:
            gstart.append(gstart[-1] + sz)
        pair2g = []
        for gi, sz in enumerate(sched):
            pair2g += [(gi, pr) for pr in range(sz // 2)]
        gp = max(sched)
        patT_pool = ctx.enter_context(tc.tile_pool(name="patT", bufs=CFG["patT_bufs"]))
        lsb_pool = ctx.enter_context(tc.tile_pool(name="lsb", bufs=CFG["lsb_bufs"]))
        attn_pool = ctx.enter_context(tc.tile_pool(name="attnp", bufs=CFG["attn_bufs"]))
        stat_pool = ctx.enter_context(tc.tile_pool(name="stats", bufs=CFG["stat_bufs"]))
        attnT_pool = ctx.enter_context(tc.tile_pool(name="attnT", bufs=CFG["attnT_bufs"]))
        vals_pool = ctx.enter_context(tc.tile_pool(name="valsp", bufs=CFG["vals_bufs"]))
        at_pool = ctx.enter_context(tc.tile_pool(name="atp", bufs=CFG["at_bufs"]))
        outp_pool = ctx.enter_context(tc.tile_pool(name="outp", bufs=CFG["outp_bufs"]))

        psumL = ctx.enter_context(tc.tile_pool(name="psL", bufs=CFG["psL"], space="PSUM"))
        psumT = ctx.enter_context(tc.tile_pool(name="psT", bufs=CFG["psT"], space="PSUM"))
        psumV = ctx.enter_context(tc.tile_pool(name="psV", bufs=CFG["psV"], space="PSUM"))
        psumVt = ctx.enter_context(tc.tile_pool(name="psVt", bufs=CFG["psVt"], space="PSUM"))
        psumA = ctx.enter_context(tc.tile_pool(name="psA", bufs=CFG["psA"], space="PSUM"))
        psumO = ctx.enter_context(tc.tile_pool(name="psO", bufs=CFG["psO"], space="PSUM"))

        # --- software-pipelined emission: per pair p, emit the PE "front"
        # (logits+vals, independent of softmax) and the softmax chain, but
        # delay the softmax-DEPENDENT PE tail (transposes/attended/out) of
        # pair p until after pair p+1's front is in the PE queue.  Engines
        # are strict FIFO, so this prevents head-of-line blocking.
        npairs = nb // 2

        def emit_loads(g):
            patTs = []
            rows = sched[g] * KP
            r0 = gstart[g] * KP
            for ec in range(EC):
                pt = patT_pool.tile([128, rows], F16, name=f"patT{ec}",
                                    tag=f"patT{ec}")
                eng = nc.scalar if ec < CFG["tp_split"] else nc.sync
                eng.dma_start(out=pt, in_=patches_h[ec, :, r0:r0 + rows])
                patTs.append(pt)
            return patTs

        group_tiles = {}

        # Lane-split pair layout: batch0 of a pair lives on partitions 0:52,
        # batch1 on partitions 64:116 (32-aligned base for matmul outputs).
        # The whole softmax chain then runs ONCE per pair on [116, K].

        def emit_front(p):
            g, pr = pair2g[p]
            if g == 0 and pr == 0:
                group_tiles[0] = emit_loads(0)
            if pr == 0 and g + 1 < len(sched):
                group_tiles[g + 1] = emit_loads(g + 1)
            patTs = group_tiles[g]
            off = pr * 2 * KP
            st = {"patTs": patTs, "off": off, "b0": p * 2}
            Lp = psumL.tile([2 * B1, K], F32)
            for ec in range(EC):
                nc.tensor.matmul(
                    Lp[0:B1, :], qkT_sb[:, ec, :], patTs[ec][:, off:off + K],
                    start=(ec == 0), stop=(ec == EC - 1),
                )
            for ec in range(EC):
                nc.tensor.matmul(
                    Lp[B1:2 * B1, :], qkT_sb[:, ec, :],
                    patTs[ec][:, off + KP:off + KP + K],
                    start=(ec == 0), stop=(ec == EC - 1),
                    skip_group_check=True,
                )
            st["Lp"] = Lp
            st["vals_sb"] = vals_pool.tile([128, 2, 2, A], F16, name="vals_sb",
                                           tag="vals_sb")
            # valsT [a, 2K] for the pair with WvT stationary (8 matmuls)
            Vtp = psumV.tile([A, 2 * K], F32)
            for ec in range(EC):
                nc.tensor.matmul(Vtp, wvT_sb[:, ec, :],
                                 patTs[ec][:, off:off + 2 * KP][:, 0:2 * K]
                                 if KP == K else patTs[ec][:, off:off + 2 * K],
                                 start=(ec == 0), stop=(ec == EC - 1))
            vT_sb = vals_pool.tile([A, 2 * K], F16, name="vT_sb", tag="vT_sb")
            nc.scalar.copy(vT_sb, Vtp)
            st["vT_sb"] = vT_sb
            return st

        def emit_chain(st):
            Lp = st["Lp"]
            lsb = lsb_pool.tile([2 * B1, K], F32)
            nc.vector.tensor_tensor(out=lsb, in0=Lp[:, 0:K], in1=cfold_sb,
                                    op=mybir.AluOpType.add)
            ssum = stat_pool.tile([2 * B1, 1], F32)
            attn_un = attn_pool.tile([2 * B1, K], F32, name="attn_un",
                                     tag="attn_un")
            nc.scalar.activation(attn_un, lsb,
                                 mybir.ActivationFunctionType.Exp,
                                 accum_out=ssum)
            rsum = stat_pool.tile([2 * B1, 1], F32)
            nc.vector.reciprocal(rsum, ssum)
            attn16 = attn_pool.tile([2 * B1, K], F16, name="attn16", tag="attn16")
            nc.vector.tensor_scalar_mul(attn16, attn_un, rsum)
            st["attn16"], st["rsum"] = attn16, rsum
            st["attn_un"] = attn_un
            attn_f32 = attn_pool.tile([2 * B1, K], F32, name="attn_f32",
                                      tag="attn_f32")
            nc.vector.tensor_scalar_mul(attn_f32, attn_un, rsum)
            nc.gpsimd.dma_start(out=attn_h[:, st["b0"] // 2, :], in_=attn_f32)
            # transpose valsT -> vals [k, a] per batch/k-chunk
            vals_sb = st["vals_sb"]
            vT_sb = st["vT_sb"]
            Tv = psumVt.tile([128, 2, 2, A], F16)
            for j in range(2):
                ko = j * K
                nc.tensor.transpose(Tv[:, j, 0, :], vT_sb[:, ko:ko + 128],
                                    ident128)
                nc.tensor.transpose(Tv[0:68, j, 1, :],
                                    vT_sb[:, ko + 128:ko + K], ident128)
                nc.scalar.copy(vals_sb[:, j, 0, :], Tv[:, j, 0, :])
                nc.vector.tensor_copy(vals_sb[0:68, j, 1, :], Tv[0:68, j, 1, :])

        def emit_tail(st):
            b0, attn16 = st["b0"], st["attn16"]
            vals_sb = st["vals_sb"]
            Tp = psumT.tile([128, 2, 2, P], F16)          # [*, j, kc, P]
            attnT_sb = attnT_pool.tile([128, 2, 2, P], F16)
            for j in range(2):
                jb = j * B1
                idn = ident[jb:jb + P, :]
                nc.tensor.transpose(Tp[:, j, 0, :],
                                    attn16[jb:jb + P, 0:128], idn)
                nc.tensor.transpose(Tp[0:68, j, 1, :],
                                    attn16[jb:jb + P, 128:196], idn)
                nc.vector.tensor_copy(attnT_sb[:, j, 0, :], Tp[:, j, 0, :])
                nc.scalar.copy(attnT_sb[0:68, j, 1, :], Tp[0:68, j, 1, :])
            Ap = psumA.tile([128, 2 * P], F32)
            for j in range(2):
                nc.tensor.matmul(Ap[:, j * P:(j + 1) * P], vals_sb[:, j, 0, :],
                                 attnT_sb[:, j, 0, :], start=True, stop=False)
                nc.tensor.matmul(Ap[:, j * P:(j + 1) * P], vals_sb[0:68, j, 1, :],
                                 attnT_sb[0:68, j, 1, :], start=False, stop=True)
            at_sb = at_pool.tile([128, 2 * P], F16)
            nc.scalar.copy(at_sb, Ap)
            Op = psumO.tile([2 * P, E], F32)
            nc.tensor.matmul(Op[:, 0:512], at_sb, woT_sb[:, 0:512])
            nc.tensor.matmul(Op[:, 512:1024], at_sb, woT_sb[:, 512:1024])
            out_sb = outp_pool.tile([2 * P, E], F32)
            nc.vector.tensor_tensor(out=out_sb[:, 0:512], in0=Op[:, 0:512],
                                    in1=bo_sb[:, 0:512], op=mybir.AluOpType.add)
            nc.vector.tensor_tensor(out=out_sb[:, 512:1024], in0=Op[:, 512:1024],
                                    in1=bo_sb[:, 512:1024], op=mybir.AluOpType.add)
            nc.gpsimd.dma_start(out=out2[b0 * P:(b0 + 2) * P, :], in_=out_sb)

        pending = None
        for p in range(npairs):
            st = emit_front(p)
            emit_chain(st)
            if pending is not None:
                emit_tail(pending)
            pending = st
        emit_tail(pending)

    if finalize:
        nc.finalize()
    return nc


def _prep_inputs(patches, queries, Wk, Wv, Wo, bo, patch_pos, temperature,
                 attn_prior, nb=NB, ncores=NCORES):
    patches = np.asarray(patches, np.float32)
    queries = np.asarray(queries, np.float32)
    Wk = np.asarray(Wk, np.float32)
    Wv = np.asarray(Wv, np.float32)
    Wo = np.asarray(Wo, np.float32)
    bo = np.asarray(bo, np.float32)
    patch_pos = np.asarray(patch_pos, np.float32)
    attn_prior = np.asarray(attn_prior, np.float32)
    t = float(np.asarray(temperature))
    temp = math.log1p(math.exp(t)) + 0.5 if t < 30 else t + 0.5

    QkT = ((queries @ Wk) / (SCALE * temp)).T            # [E, P]
    QkT64 = np.zeros((E, B1), np.float32)                # pad P=52 -> 64 cols
    QkT64[:, :P] = QkT
    qkT_c = np.ascontiguousarray(
        QkT64.reshape(EC, 128, B1).transpose(1, 0, 2)).astype(np.float16)
    WvT = Wv.T                                            # [E, A]
    wvT_c = np.ascontiguousarray(
        WvT.reshape(EC, 128, A).transpose(1, 0, 2)).astype(np.float16)
    woT = np.ascontiguousarray(Wo.T).astype(np.float16)   # [A, E]
    cf = ((queries @ patch_pos.T) / SCALE + attn_prior) / temp  # [P, K]
    cfold = np.zeros((2 * B1, K), np.float32)                   # lane-split dup
    cfold[:P] = cf
    cfold[B1:B1 + P] = cf
    bo2 = np.ascontiguousarray(np.tile(bo[None, :], (2 * P, 1)))

    nuse = nb * ncores
    p16 = patches[:nuse].astype(np.float16)
    # host pre-transpose into the exact per-chunk SBUF layout:
    # [core, EC, 128(e-in-chunk), nb*K]
    shards = np.ascontiguousarray(
        p16.reshape(ncores, nb, KP, EC, 128).transpose(0, 3, 4, 1, 2)
    ).reshape(ncores, EC, 128, nb * KP)

    in_maps = []
    for c in range(ncores):
        in_maps.append({
            "patches16": shards[c],
            "qkT": qkT_c, "wvT": wvT_c, "woT": woT,
            "cfold": cfold, "bo2": bo2,
        })
    return in_maps


def _get_module(nb=NB):
    key = (nb, tuple(sorted(CFG.items())))
    if key not in _cache:
        _cache[key] = build_module(nb)
    return _cache[key]


def _run(inputs, trace=False, nb=NB):
    from concourse.bass_utils import run_bass_kernel_spmd
    nc = _get_module(nb)
    in_maps = _prep_inputs(**inputs, nb=nb)
    res = run_bass_kernel_spmd(nc, in_maps, core_ids=list(range(NCORES)),
                               trace=trace)
    outs = np.concatenate([r["out"] for r in res.results], axis=0)
    attns = np.concatenate([_attn_fix(r["attn"]) for r in res.results], axis=0)
    return (outs, attns), res


def _attn_fix(a):
    # a: [128, nb//2, K] lane-split pairs -> [nb, P, K]
    npr = a.shape[1]
    out = np.empty((npr * 2, P, K), a.dtype)
    out[0::2] = a[0:P].transpose(1, 0, 2)
    out[1::2] = a[B1:B1 + P].transpose(1, 0, 2)
    return out


def kernel(**inputs):
    (outs, attns), _ = _run(inputs, trace=False)
    return outs, attns
